# revision 36
# baseline (speedup 1.0000x reference)
"""MoE gate (router) kernel for Trainium2, 8 NeuronCores, data-parallel.

reference: logits = x @ W_g  ([16384,2048] @ [2048,64]); scores = softmax(logits);
           return top-6 (indices, scores).

Strategy
--------
Data-parallel over tokens: each of the 8 cores handles 2048 tokens. The
contraction dim K=2048 must live on SBUF partitions, so the host feeds each
core pre-transposed contiguous shards. fp32 matmul on PE is ~4x slow with
un-hidden fp32 LDWEIGHTS, so x and W are split on the host into fp16 hi/lo
pairs (lo scaled by 2^11 to stay in fp16 normal range):

    x = xh + 2^-11 * xl',   W = Wh + 2^-11 * Wl'     (all fp16, exact split)
    logits = xh@Wh + 2^-11 * (xh@Wl' + xl'@Wh)  (+ 2^-22 xl'@Wl', dropped)

All fp16 products are exact in fp32 PSUM accumulation -> ~2^-22 relative
logit error, i.e. fp32-class. PE runs at full rate with fast weight load.

Per core:
  - Ws = [Wh | Wl'] [2048, 128] fp16 resident in SBUF (16 chunks of [128,128]).
  - Tokens in blocks of 1024 (2 KiB DMA segments per partition).
  - Per 128-token tile: group A = 16 matmuls xh-chunk @ Ws-chunk (N=128) into
    PSUM [128,128]; group B = 16 matmuls xl'-chunk @ Wh-chunk (N=64) into
    PSUM [128,64].
  - Fold (ACT copy + 2 DVE ops): lg = A[:,0:64] + 2^-11*(A[:,64:128] + B).
  - Softmax+top6 without max-subtraction (|logits| < ~6, exp is safe):
    erow=exp(lg) with ACT accumulate -> sume; max8/find_index8 on raw lg;
    scores = exp(v6) * (1/sume).
  - Outputs staged in SBUF; one DMA per output at the end.
"""

import os
import sys

import numpy as np

for _p in ("/opt/trn_rl_repo", "/root/.axon_site/_ro/trn_rl_repo"):
    if os.path.isdir(_p) and _p not in sys.path:
        sys.path.insert(0, _p)

import concourse.bass as bass
import concourse.mybir as mybir
from concourse import bacc, bass_utils
from concourse.tile import TileContext

N_CORES = 8
T_FULL = 16384
K = 2048
E = 64
TOPK = 6
P = 128
LO_SCALE = 2048.0  # 2^11

_NC_CACHE: dict[int, "bass.Bass"] = {}
LAST_RESULT = None  # BassKernelResults of the most recent kernel() call


def _blocks_for(t_shard: int) -> list[int]:
    if t_shard == T_FULL // N_CORES:
        return [512, 512, 512, 256, 256]
    blocks = []
    rem = t_shard
    while rem > 0:
        b = min(512, rem)
        blocks.append(b)
        rem -= b
    return blocks


def build_nc(t_shard: int = T_FULL // N_CORES) -> "bass.Bass":
    f16 = mybir.dt.float16
    f32 = mybir.dt.float32
    i32 = mybir.dt.int32
    u32 = mybir.dt.uint32
    EXP = mybir.ActivationFunctionType.Exp

    kc = K // P  # 16 contraction chunks
    assert t_shard % P == 0
    nt = t_shard // P  # total 128-token tiles
    blocks = _blocks_for(t_shard)
    assert sum(blocks) == t_shard

    nc = bacc.Bacc()
    # packed layout: per K-row, each token block stores [hi(tblk) | lo(tblk)]
    # contiguously -> one 2*tblk*2B contiguous DRAM segment per partition per
    # chunk DMA (DMA engines are packet-rate bound; bigger packets = more BW).
    xhl = nc.dram_tensor("xhl", [K, 2 * t_shard], f16, kind="ExternalInput")
    Ws = nc.dram_tensor("Ws", [K, 2 * E], f16, kind="ExternalInput")
    # outputs in partition-major layout [P, nt, TOPK]; host reorders.
    idx_o = nc.dram_tensor("idx", [P, nt, TOPK], i32, kind="ExternalOutput")
    val_o = nc.dram_tensor("val", [P, nt, TOPK], f32, kind="ExternalOutput")

    with TileContext(nc) as tc:
        with (
            tc.tile_pool(name="singles", bufs=1) as singles,
            tc.tile_pool(name="xpool", bufs=3) as xpool,
            tc.tile_pool(name="small", bufs=4) as small,
            tc.tile_pool(name="psum", bufs=4, space="PSUM") as psum_pool,
        ):
            W_sb = singles.tile([P, kc, 2 * E], f16)
            Wr = Ws[:].rearrange("(c p) e -> p c e", p=P)
            for c in range(kc):
                eng = (nc.sync, nc.gpsimd)[c % 2]
                eng.dma_start(out=W_sb[:, c], in_=Wr[:, c])
            vstage = singles.tile([P, nt, TOPK], f32)
            istage = singles.tile([P, nt, TOPK], i32)

            t0 = 0
            for b, tblk in enumerate(blocks):
                spt = tblk // P
                xbs = []
                for c in range(kc):
                    # Spread descriptor-generation cost: the sequencer pays
                    # ~0.65us per dma_start, so a single engine cannot keep
                    # 16 queues fed. PE helps only on block 0 (it is idle).
                    # ~0.65us of sequencer time per dma_start regardless of
                    # size: one engine cannot issue the whole stream. Spread
                    # across the three DMA-capable engines.
                    if b == 0:
                        eng = (nc.sync, nc.gpsimd)[c % 2]
                    else:
                        eng = (nc.sync, nc.scalar, nc.gpsimd)[c % 3]
                    xb = xpool.tile([P, 2, tblk], f16, tag=f"x{c}")
                    eng.dma_start(
                        out=xb,
                        in_=xhl[
                            c * P : (c + 1) * P, 2 * t0 : 2 * t0 + 2 * tblk
                        ].rearrange("p (h t) -> p h t", h=2),
                    )
                    xbs.append(xb)
                for s in range(spt):
                    tok = slice(s * P, (s + 1) * P)
                    psA = psum_pool.tile([P, 2 * E], f32, tag="psA")
                    psB = psum_pool.tile([P, E], f32, tag="psB")
                    for c in range(kc):
                        nc.tensor.matmul(
                            psA,
                            xbs[c][:, 0, tok],
                            W_sb[:, c],
                            start=(c == 0),
                            stop=(c == kc - 1),
                        )
                    for c in range(kc):
                        nc.tensor.matmul(
                            psB,
                            xbs[c][:, 1, tok],
                            W_sb[:, c, :E],
                            start=(c == 0),
                            stop=(c == kc - 1),
                        )
                    # fold: lg = A[:, :64] + 2^-11 * (A[:, 64:] + B)
                    t1 = small.tile([P, E], f32, tag="t1")
                    nc.vector.tensor_copy(t1, psB)
                    t2 = small.tile([P, E], f32, tag="t2")
                    nc.vector.tensor_add(t2, psA[:, E:], t1)
                    lg = small.tile([P, E], f32, tag="lg")
                    nc.vector.scalar_tensor_tensor(
                        out=lg,
                        in0=t2,
                        scalar=1.0 / LO_SCALE,
                        in1=psA[:, :E],
                        op0=mybir.AluOpType.mult,
                        op1=mybir.AluOpType.add,
                    )
                    # softmax + top-6 (no max subtraction; |logits| < ~6)
                    erow = small.tile([P, E], f32, tag="erow")
                    sume = small.tile([P, 1], f32, tag="sume")
                    nc.scalar.activation(erow, lg, EXP)
                    nc.vector.tensor_reduce(
                        sume, erow, axis=mybir.AxisListType.X, op=mybir.AluOpType.add
                    )
                    v8 = small.tile([P, 8], f32, tag="v8")
                    nc.vector.max(out=v8, in_=lg)
                    i8 = small.tile([P, 8], u32, tag="i8")
                    nc.vector.max_index(out=i8, in_max=v8, in_values=lg)
                    rec = small.tile([P, 1], f32, tag="rec")
                    nc.vector.reciprocal(rec, sume)
                    ev = small.tile([P, TOPK], f32, tag="ev")
                    nc.scalar.activation(ev, v8[:, :TOPK], EXP)
                    tt = t0 // P + s
                    nc.vector.tensor_scalar_mul(vstage[:, tt], ev, rec)
                    nc.vector.tensor_copy(istage[:, tt], i8[:, :TOPK])
                t0 += tblk
            nc.sync.dma_start(out=idx_o[:], in_=istage)
            nc.scalar.dma_start(out=val_o[:], in_=vstage)
    if not nc.is_finalized():
        nc.finalize()
    return nc


def _get_nc(t_shard: int) -> "bass.Bass":
    if t_shard not in _NC_CACHE:
        _NC_CACHE[t_shard] = build_nc(t_shard)
    return _NC_CACHE[t_shard]


def _split_hi_lo(a: np.ndarray) -> tuple[np.ndarray, np.ndarray]:
    hi = a.astype(np.float16)
    lo = ((a - hi.astype(np.float32)) * np.float32(LO_SCALE)).astype(np.float16)
    return hi, lo


def kernel(x: np.ndarray, W_g: np.ndarray, **run_kwargs):
    global LAST_RESULT
    x = np.asarray(x, dtype=np.float32)
    W = np.asarray(W_g, dtype=np.float32)
    t_shard = x.shape[0] // N_CORES
    nc = _get_nc(t_shard)

    xh, xl = _split_hi_lo(x)
    Wh, Wl = _split_hi_lo(W)
    Ws = np.ascontiguousarray(np.concatenate([Wh, Wl], axis=1))
    xhT = xh.T  # [K, T]
    xlT = xl.T
    blocks = _blocks_for(t_shard)
    in_maps = []
    for cix in range(N_CORES):
        xp = np.empty((K, 2 * t_shard), np.float16)
        t0 = cix * t_shard
        off = 0
        for tblk in blocks:
            xp[:, off : off + tblk] = xhT[:, t0 : t0 + tblk]
            xp[:, off + tblk : off + 2 * tblk] = xlT[:, t0 : t0 + tblk]
            off += 2 * tblk
            t0 += tblk
        in_maps.append({"xhl": xp, "Ws": Ws})
    res = bass_utils.run_bass_kernel_spmd(
        nc, in_maps, core_ids=list(range(N_CORES)), **run_kwargs
    )
    LAST_RESULT = res
    # device layout is [P, nt, TOPK]; token t = tile*P + p -> [t_shard, TOPK]
    idx = np.concatenate(
        [np.moveaxis(r["idx"], 0, 1).reshape(t_shard, TOPK) for r in res.results],
        axis=0,
    ).astype(np.int32)
    val = np.concatenate(
        [np.moveaxis(r["val"], 0, 1).reshape(t_shard, TOPK) for r in res.results],
        axis=0,
    ).astype(np.float32)
    return idx, val


# revision 41
# speedup vs baseline: 1.0900x; 1.0900x over previous
"""MoE gate (router) kernel for Trainium2, 8 NeuronCores, data-parallel.

reference: logits = x @ W_g  ([16384,2048] @ [2048,64]); scores = softmax(logits);
           return top-6 (indices, scores).

Strategy
--------
Data-parallel over tokens: each of the 8 cores handles 2048 tokens. The
contraction dim K=2048 must live on SBUF partitions, so the host feeds each
core pre-transposed contiguous shards. fp32 matmul on PE is ~4x slow with
un-hidden fp32 LDWEIGHTS, so x and W are split on the host into fp16 hi/lo
pairs (lo scaled by 2^11 to stay in fp16 normal range):

    x = xh + 2^-11 * xl',   W = Wh + 2^-11 * Wl'     (all fp16, exact split)
    logits = xh@Wh + 2^-11 * (xh@Wl' + xl'@Wh)  (+ 2^-22 xl'@Wl', dropped)

All fp16 products are exact in fp32 PSUM accumulation -> ~2^-22 relative
logit error, i.e. fp32-class. PE runs at full rate with fast weight load.

Per core:
  - Ws = [Wh | Wl'] [2048, 128] fp16 resident in SBUF (16 chunks of [128,128]).
  - DRAM layout of x is packed per 512-token block as [hi(tblk) | lo(tblk)]
    per K-row, so every DMA moves 2KiB-contiguous per-partition segments
    (the SDMA engines are packet-rate bound; small packets cannot reach HBM
    bandwidth).
  - All blocks but the last: ONE dma_start per block (a single dma_start is
    split across all 16 SDMA engines; >=1MiB reaches max bandwidth), then per
    128-token subtile: group A = 16 matmuls xh@[Wh|Wl'] (N=128) and group
    B = 16 matmuls xl'@Wh (N=64) accumulated in PSUM.
  - Last block: 16 per-chunk DMAs on the same FIFO ring (arrivals stagger),
    chunk-outer matmul order across 8 open PSUM banks -> PE trails the tail
    of the stream by one chunk instead of one block.
  - Fold (3 DVE ops): lg = A[:,0:64] + 2^-11*(A[:,64:128] + B).
  - Softmax+top6 without max-subtraction (|logits| < ~6): erow=exp(lg) (ACT),
    sume (DVE reduce), max8/find_index8 on raw logits, scores=exp(v6)/sume.
  - Outputs staged in SBUF [128, 16, 6] (partition-major; host reorders),
    single DMA per output at the end.
"""

import os
import sys

import numpy as np

for _p in ("/opt/trn_rl_repo", "/root/.axon_site/_ro/trn_rl_repo"):
    if os.path.isdir(_p) and _p not in sys.path:
        sys.path.insert(0, _p)

import concourse.bass as bass
import concourse.mybir as mybir
from concourse import bacc, bass_utils
from concourse.tile import TileContext

N_CORES = 8
T_FULL = 16384
K = 2048
E = 64
TOPK = 6
P = 128
LO_SCALE = 2048.0  # 2^11

_NC_CACHE: dict[int, "bass.Bass"] = {}
LAST_RESULT = None  # BassKernelResults of the most recent kernel() call


def _blocks_for(t_shard: int) -> list[int]:
    blocks = []
    rem = t_shard
    while rem > 0:
        b = min(512, rem)
        blocks.append(b)
        rem -= b
    return blocks


def build_nc(t_shard: int = T_FULL // N_CORES) -> "bass.Bass":
    f16 = mybir.dt.float16
    f32 = mybir.dt.float32
    i32 = mybir.dt.int32
    u32 = mybir.dt.uint32
    EXP = mybir.ActivationFunctionType.Exp

    kc = K // P  # 16 contraction chunks
    assert t_shard % P == 0
    nt = t_shard // P  # total 128-token tiles
    blocks = _blocks_for(t_shard)

    nc = bacc.Bacc()
    # packed: per K-row, token block b stores [hi(tblk_b) | lo(tblk_b)]
    # contiguously (2KiB per partition per chunk for tblk=512)
    xhl = nc.dram_tensor("xhl", [K, 2 * t_shard], f16, kind="ExternalInput")
    Ws = nc.dram_tensor("Ws", [K, 2 * E], f16, kind="ExternalInput")
    # outputs in partition-major layout [P, nt, TOPK]; host reorders.
    idx_o = nc.dram_tensor("idx", [P, nt, TOPK], i32, kind="ExternalOutput")
    val_o = nc.dram_tensor("val", [P, nt, TOPK], f32, kind="ExternalOutput")

    with TileContext(nc) as tc:
        with (
            tc.tile_pool(name="singles", bufs=1) as singles,
            tc.tile_pool(name="xpool", bufs=3) as xpool,
            tc.tile_pool(name="small", bufs=4) as small,
            tc.tile_pool(name="psum", bufs=4, space="PSUM") as psum_pool,
        ):
            W_sb = singles.tile([P, kc, 2 * E], f16)
            nc.scalar.dma_start(
                out=W_sb, in_=Ws[:].rearrange("(c p) e -> p c e", p=P)
            )
            vstage = singles.tile([P, nt, TOPK], f32)
            istage = singles.tile([P, nt, TOPK], i32)

            def softmax_top6(psA, psB, tt):
                # fold: lg = A[:, :64] + 2^-11 * (A[:, 64:] + B)
                t1 = small.tile([P, E], f32, tag="t1")
                nc.vector.tensor_copy(t1, psB)
                t2 = small.tile([P, E], f32, tag="t2")
                nc.vector.tensor_add(t2, psA[:, E:], t1)
                lg = small.tile([P, E], f32, tag="lg")
                nc.vector.scalar_tensor_tensor(
                    out=lg,
                    in0=t2,
                    scalar=1.0 / LO_SCALE,
                    in1=psA[:, :E],
                    op0=mybir.AluOpType.mult,
                    op1=mybir.AluOpType.add,
                )
                # softmax + top-6 (no max subtraction; |logits| < ~6)
                erow = small.tile([P, E], f32, tag="erow")
                sume = small.tile([P, 1], f32, tag="sume")
                nc.scalar.activation(erow, lg, EXP)
                nc.vector.tensor_reduce(
                    sume, erow, axis=mybir.AxisListType.X, op=mybir.AluOpType.add
                )
                v8 = small.tile([P, 8], f32, tag="v8")
                nc.vector.max(out=v8, in_=lg)
                i8 = small.tile([P, 8], u32, tag="i8")
                nc.vector.max_index(out=i8, in_max=v8, in_values=lg)
                rec = small.tile([P, 1], f32, tag="rec")
                nc.vector.reciprocal(rec, sume)
                ev = small.tile([P, TOPK], f32, tag="ev")
                nc.scalar.activation(ev, v8[:, :TOPK], EXP)
                nc.vector.tensor_scalar_mul(vstage[:, tt], ev, rec)
                nc.vector.tensor_copy(istage[:, tt], i8[:, :TOPK])

            t0 = 0
            for tblk in blocks[:-1]:
                spt = tblk // P
                xb = xpool.tile([P, kc, 2 * tblk], f16, tag="xbig")
                nc.sync.dma_start(
                    out=xb,
                    in_=xhl[:, 2 * t0 : 2 * t0 + 2 * tblk].rearrange(
                        "(c p) w -> p c w", p=P
                    ),
                )
                for s in range(spt):
                    tokh = slice(s * P, (s + 1) * P)
                    tokl = slice(tblk + s * P, tblk + (s + 1) * P)
                    psA = psum_pool.tile([P, 2 * E], f32, tag="psA")
                    psB = psum_pool.tile([P, E], f32, tag="psB")
                    for c in range(kc):
                        nc.tensor.matmul(
                            psA,
                            xb[:, c, tokh],
                            W_sb[:, c],
                            start=(c == 0),
                            stop=(c == kc - 1),
                        )
                    for c in range(kc):
                        nc.tensor.matmul(
                            psB,
                            xb[:, c, tokl],
                            W_sb[:, c, :E],
                            start=(c == 0),
                            stop=(c == kc - 1),
                        )
                    softmax_top6(psA, psB, t0 // P + s)
                t0 += tblk

            # last block: per-chunk DMAs + chunk-outer matmuls for a short tail
            tblk = blocks[-1]
            spt = tblk // P
            xcs = []
            for c in range(kc):
                xc = xpool.tile([P, 2, tblk], f16, tag=f"xc{c}")
                nc.sync.dma_start(
                    out=xc,
                    in_=xhl[
                        c * P : (c + 1) * P, 2 * t0 : 2 * t0 + 2 * tblk
                    ].rearrange("p (h t) -> p h t", h=2),
                )
                xcs.append(xc)
            psAs = [
                psum_pool.tile([P, 2 * E], f32, tag="psA", name=f"psAL{s}")
                for s in range(spt)
            ]
            psBs = [
                psum_pool.tile([P, E], f32, tag="psB", name=f"psBL{s}")
                for s in range(spt)
            ]
            for c in range(kc):
                for s in range(spt):
                    nc.tensor.matmul(
                        psAs[s],
                        xcs[c][:, 0, s * P : (s + 1) * P],
                        W_sb[:, c],
                        start=(c == 0),
                        stop=(c == kc - 1),
                    )
                for s in range(spt):
                    nc.tensor.matmul(
                        psBs[s],
                        xcs[c][:, 1, s * P : (s + 1) * P],
                        W_sb[:, c, :E],
                        start=(c == 0),
                        stop=(c == kc - 1),
                    )
            for s in range(spt):
                softmax_top6(psAs[s], psBs[s], t0 // P + s)

            nc.sync.dma_start(out=idx_o[:], in_=istage)
            nc.scalar.dma_start(out=val_o[:], in_=vstage)
    if not nc.is_finalized():
        nc.finalize()
    return nc


def _get_nc(t_shard: int) -> "bass.Bass":
    if t_shard not in _NC_CACHE:
        _NC_CACHE[t_shard] = build_nc(t_shard)
    return _NC_CACHE[t_shard]


def _split_hi_lo(a: np.ndarray) -> tuple[np.ndarray, np.ndarray]:
    hi = a.astype(np.float16)
    lo = ((a - hi.astype(np.float32)) * np.float32(LO_SCALE)).astype(np.float16)
    return hi, lo


def pack_core_input(xhT, xlT, t0: int, t_shard: int) -> np.ndarray:
    xp = np.empty((K, 2 * t_shard), np.float16)
    off = 0
    for tblk in _blocks_for(t_shard):
        xp[:, off : off + tblk] = xhT[:, t0 : t0 + tblk]
        xp[:, off + tblk : off + 2 * tblk] = xlT[:, t0 : t0 + tblk]
        off += 2 * tblk
        t0 += tblk
    return xp


def kernel(x: np.ndarray, W_g: np.ndarray, **run_kwargs):
    global LAST_RESULT
    x = np.asarray(x, dtype=np.float32)
    W = np.asarray(W_g, dtype=np.float32)
    t_shard = x.shape[0] // N_CORES
    nc = _get_nc(t_shard)

    xh, xl = _split_hi_lo(x)
    Wh, Wl = _split_hi_lo(W)
    Ws = np.ascontiguousarray(np.concatenate([Wh, Wl], axis=1))
    xhT = xh.T  # [K, T]
    xlT = xl.T
    in_maps = [
        {"xhl": pack_core_input(xhT, xlT, c * t_shard, t_shard), "Ws": Ws}
        for c in range(N_CORES)
    ]
    res = bass_utils.run_bass_kernel_spmd(
        nc, in_maps, core_ids=list(range(N_CORES)), **run_kwargs
    )
    LAST_RESULT = res
    # device layout is [P, nt, TOPK]; token t = tile*P + p -> [t_shard, TOPK]
    idx = np.concatenate(
        [np.moveaxis(r["idx"], 0, 1).reshape(t_shard, TOPK) for r in res.results],
        axis=0,
    ).astype(np.int32)
    val = np.concatenate(
        [np.moveaxis(r["val"], 0, 1).reshape(t_shard, TOPK) for r in res.results],
        axis=0,
    ).astype(np.float32)
    return idx, val
